# revision 8
# baseline (speedup 1.0000x reference)
"""Trainium2 Bass kernel for nn_CentersDistance (retrieval_knn).

logits[k, n] = -||centers[k] - inputs[n]||^2
             = 2*(centers @ inputs.T)[k, n] - ||centers[k]||^2 - ||inputs[n]||^2

Strategy (8 NeuronCores, data-parallel over the N=8192 inputs):
  * host: transpose both operands so the contraction dim D lands on the SBUF
    partition axis, fold the factor 2 into the inputs, quantize both to
    fp8e4m3 (TRN float8e4), and precompute the norm terms exactly in float64.
  * device (per core): a 1024x1024x1024 matmul in fp8 with DoubleRow perf
    mode: each InstMatmult consumes TWO 128-deep contraction tiles laid out
    as [128, 2, free] (2 rows/cycle on the PE = 157 TF/s, 2x the bf16 rate),
    so the whole GEMM is 64 matmul instructions instead of 128.  PSUM
    accumulation stays fp32.
  * the PSUM->SBUF epilogue (add -||c||^2 per-partition and -||x||^2
    broadcast row, emit bf16) is the serial tail bottleneck (~740ns per
    [128, 512] group on the DVE), so it is split across THREE engines:
      - even groups: DVE scalar_tensor_tensor (one op per group);
      - odd groups: Activation engine adds the per-partition -||c||^2 via
        an Identity-activation bias read straight from PSUM (this also
        frees the PSUM bank for pass 2), then GpSimd adds the -||x||^2 row
        (tensor_add, SBUF-only - GpSimd cannot touch PSUM).
  * -||x||^2 is shipped as a single [1, 1024] fp32 row (4 KB) and broadcast
    to all 128 partitions on-chip by GpSimd (partition_broadcast), replacing
    the baseline's 512 KB host-broadcast load.
  * raw Block/semaphore implementation (not Tile), same skeleton as the
    bf16 baseline: two HW-DGE queues (Sync: xt, Scalar: ct) with one
    semaphore per d-pair; PE warmup matmuls bridge the NRT preamble until
    the first tile pair lands (~10.5us) and keep the HAM clock ramp alive
    (an idle PE resets it - observed: a 4us tile stall restarted the ramp
    and cost 13 matmuls at 1.2GHz); pass 1 (m-tiles 0-3) runs d outermost
    to pace with the streaming loads across 8 PSUM banks; pass 2 (m-tiles
    4-7) runs d innermost so each output group retires early and its
    epilogue + store overlap the remaining matmuls.
  * stores pair adjacent groups (same m-tile -> contiguous in out) into
    single [128, 1024] bf16 DMAs (2 KB/partition lines) alternating between
    the two queues; the last group is split in half across both queues to
    shorten the tail.  Output is bf16 (halves store traffic vs fp32; host
    converts back).

Accuracy: the exact f64 norm terms dominate the logits; fp8 cross term +
bf16 store measured absmax/scale 7.3e-3 vs the 2e-2 gate (bf16 baseline:
3.3e-4 at 45us, kept in kernel_bf16_baseline.py as fallback).
"""

import threading
from contextlib import ExitStack

import numpy as np
import ml_dtypes

import concourse.mybir as mybir
from concourse import bacc
from concourse.bass_utils import run_bass_kernel_spmd

N_CORES = 8
N, K, D = 8192, 1024, 1024
NSH = N // N_CORES  # per-core slab of inputs
P = 128             # SBUF partitions
NF = 512            # matmul moving free dim (one fp32 PSUM bank)

DP_TILES = D // (2 * P)  # 4 double-row contraction tiles (256 deep each)
M_TILES = K // P         # 8 center tiles
H_TILES = NSH // NF      # 2 moving-dim tiles

G = M_TILES * H_TILES  # 16 output groups of [128, 512]
GP1 = 8                # groups 0-7 -> pass 1 (m-tiles 0-3), banks 0-7
N_WU = 10              # PE warm-up matmuls (bridge preamble-end -> first tile)
N_TMP = 4              # fp32 staging buffers for the Act->GpSimd odd path

_DT = mybir.dt.float8e4
_NP_DT = ml_dtypes.float8_e4m3
_OUT_DT = mybir.dt.float16
_DR = mybir.MatmulPerfMode.DoubleRow

_cache = threading.local()


def _g_mh(g):
    return g // H_TILES, g % H_TILES


def _build_nc():
    nc = bacc.Bacc(
        "TRN2", target_bir_lowering=False, debug=False, num_devices=N_CORES
    )
    ct = nc.dram_tensor("ct", [DP_TILES, P, 2, K], _DT, kind="ExternalInput").ap()
    xt = nc.dram_tensor("xt", [DP_TILES, P, 2, NSH], _DT, kind="ExternalInput").ap()
    ncsq = nc.dram_tensor(
        "ncsq", [P, M_TILES], mybir.dt.float32, kind="ExternalInput"
    ).ap()
    nxrow = nc.dram_tensor(
        "nxrow", [1, NSH], mybir.dt.float16, kind="ExternalInput"
    ).ap()
    out = nc.dram_tensor("out", [K, NSH], _OUT_DT, kind="ExternalOutput").ap()

    out_r = out.rearrange("(m p) n -> m p n", p=P)

    HNF = NF // 2
    ODD = list(range(1, G - 1, 2))  # odd groups 1..13 via Act+GpSimd
    # group 15 is drained in two halves, also via Act+GpSimd (tail latency)

    with (
        nc.sbuf_tensor("wu_sb", [P, 2, NF], _DT) as wu_sb,
        nc.sbuf_tensor("ncsq_sb", [P, M_TILES], mybir.dt.float32) as ncsq_sb,
        nc.sbuf_tensor("nxrow_sb", [1, NSH], mybir.dt.float16) as nxrow_sb,
        nc.sbuf_tensor("nxsq_sb", [P, NSH], mybir.dt.float16) as nxsq_sb,
        nc.sbuf_tensor("tmp_sb", [P, N_TMP, NF], mybir.dt.float16) as tmp_sb,
        nc.sbuf_tensor("ot_sb", [P, G * NF], _OUT_DT) as ot_sb,
        ExitStack() as stack,
        nc.semaphore("row_sem") as row_sem,
        nc.semaphore("const_sem") as const_sem,
        nc.semaphore("bc_sem") as bc_sem,
        nc.semaphore("mm_sem") as mm_sem,
        nc.semaphore("dve_e") as dve_e,   # even group drained+done (DVE)
        nc.semaphore("ob_sem") as ob_sem, # odd group PSUM drained (Act)
        nc.semaphore("od_sem") as od_sem, # odd group data ready (GpSimd)
        nc.semaphore("dma_out") as dma_out,
        nc.Block() as block,
    ):
        d_sems = [
            stack.enter_context(nc.semaphore(f"d_sem{i}")) for i in range(DP_TILES)
        ]
        ct_sb = [
            stack.enter_context(nc.sbuf_tensor(f"ct_sb{d}", [P, 2, K], _DT))
            for d in range(DP_TILES)
        ]
        xt_sb = [
            stack.enter_context(nc.sbuf_tensor(f"xt_sb{d}", [P, 2, NSH], _DT))
            for d in range(DP_TILES)
        ]
        ps = [
            stack.enter_context(nc.psum_tensor(f"ps{b}", [P, NF], mybir.dt.float32))
            for b in range(8)
        ]

        # store schedule: pair adjacent groups (same m-tile -> contiguous in
        # out) into one [128, 1024] bf16 DMA with 2KB/partition lines.
        # Sync queue: pairs 0,2,4,6 + group 14 + first half of group 15.
        # Scalar queue: pairs 1,3,5 + second half of group 15.
        def pair_store(eng, gp):
            g0 = 2 * gp
            m, _ = _g_mh(g0)
            eng.wait_ge(dve_e, gp + 1)
            eng.wait_ge(od_sem, gp + 1)
            eng.dma_start(
                out_r[m][:],
                ot_sb[:, g0 * NF : (g0 + 2) * NF],
            ).then_inc(dma_out, 16)

        N_STORES = 7 + 1 + 2  # 7 pairs + group 14 + two halves of group 15

        @block.sync
        def _(sync):
            # consts first: 6KB, and the GpSimd broadcast wants nxrow early
            sync.dma_start(nxrow_sb[:], nxrow).then_inc(row_sem, 16)
            sync.dma_start(ncsq_sb[:], ncsq).then_inc(const_sem, 16)
            # xt tiles 0-2 on the Sync HW-DGE queue; ct 0-2 in parallel on
            # the Scalar queue; the dp3 pair rides GpSimd's software-DGE
            # ring as a third HBM stream (loads are the critical resource:
            # the pass-1 stop round cannot run until the last tile lands)
            for d in range(DP_TILES - 1):
                sync.dma_start(xt_sb[d][:], xt[d]).then_inc(d_sems[d], 16)
            for gp in (0, 2, 4, 6):
                pair_store(sync, gp)
            # group 14 whole
            m, h = _g_mh(G - 2)
            sync.wait_ge(dve_e, 8)
            sync.dma_start(
                out_r[m][:, h * NF : (h + 1) * NF],
                ot_sb[:, (G - 2) * NF : (G - 1) * NF],
            ).then_inc(dma_out, 16)
            # first half of group 15
            m, h = _g_mh(G - 1)
            sync.wait_ge(od_sem, 8)
            sync.dma_start(
                out_r[m][:, h * NF : h * NF + HNF],
                ot_sb[:, (G - 1) * NF : (G - 1) * NF + HNF],
            ).then_inc(dma_out, 16)
            sync.wait_ge(dma_out, N_STORES * 16)

        @block.scalar
        def _(scalar):
            for d in range(DP_TILES - 1):
                scalar.dma_start(ct_sb[d][:], ct[d]).then_inc(d_sems[d], 16)
            # odd-group epilogue step 1 on the Activation engine:
            # tmp = Identity(ps + ncsq_bias).  Reads PSUM (which GpSimd
            # cannot), so this is also what frees the bank for pass 2.
            scalar.wait_ge(const_sem, 16)  # ncsq present
            n_act = 0

            def act_drain(g, lo, hi):
                nonlocal n_act
                buf = n_act % N_TMP
                if n_act >= N_TMP:
                    # buffer reuse: GpSimd must have consumed its previous
                    # occupant
                    scalar.wait_ge(od_sem, n_act - N_TMP + 1)
                m, _ = _g_mh(g)
                nc.scalar.add(
                    tmp_sb[:, buf, 0 : hi - lo],
                    ps[g % 8][:, lo:hi],
                    ncsq_sb[:, m : m + 1],
                ).then_inc(ob_sem, 1)
                n_act += 1

            # interleave the Scalar-queue stores so they dispatch as soon
            # as their groups are ready without stalling later act ops
            for j, g in enumerate(ODD):
                scalar.wait_ge(mm_sem, g + 1)
                act_drain(g, 0, NF)
                if g == 5:
                    pair_store(scalar, 1)   # groups (2,3)
                elif g == 9:
                    pair_store(scalar, 3)   # groups (6,7)
                elif g == 13:
                    pair_store(scalar, 5)   # groups (10,11)
            # group 15 in two halves
            scalar.wait_ge(mm_sem, G)
            act_drain(G - 1, 0, HNF)
            act_drain(G - 1, HNF, NF)
            # second half of group 15
            m, h = _g_mh(G - 1)
            scalar.wait_ge(od_sem, 9)
            scalar.dma_start(
                out_r[m][:, h * NF + HNF : (h + 1) * NF],
                ot_sb[:, (G - 1) * NF + HNF : G * NF],
            ).then_inc(dma_out, 16)

        @block.gpsimd
        def _(gpsimd):
            d3 = DP_TILES - 1
            gpsimd.dma_start(xt_sb[d3][:], xt[d3]).then_inc(d_sems[d3], 16)
            gpsimd.dma_start(ct_sb[d3][:], ct[d3]).then_inc(d_sems[d3], 16)
            gpsimd.wait_ge(row_sem, 16)
            nc.gpsimd.partition_broadcast(nxsq_sb[:], nxrow_sb[:]).then_inc(
                bc_sem, 1
            )

        @block.tensor
        def _(tensor):
            # warm-up: keep the PE busy (and the HAM clock ramp alive) from
            # preamble-end until the first ct/xt tile pair lands.  wu_sb is
            # deliberately uninitialized - the products are never read.
            # Bank 7 is rewritten with start=True by group 7's first matmul
            # ~8 matmuls later.
            for _ in range(N_WU):
                nc.tensor.matmul(
                    ps[GP1 - 1][:],
                    wu_sb[:, :, 0:P],
                    wu_sb[:, :, :],
                    start=True,
                    stop=True,
                    perf_mode=_DR,
                )
            # pass 1: groups 0-7 accumulate in banks 0-7, d outermost so
            # matmuls pace with the streaming loads
            for d in range(DP_TILES):
                tensor.wait_ge(d_sems[d], 32)
                for g in range(GP1):
                    m, h = _g_mh(g)
                    mm = nc.tensor.matmul(
                        ps[g][:],
                        ct_sb[d][:, :, m * P : (m + 1) * P],
                        xt_sb[d][:, :, h * NF : (h + 1) * NF],
                        start=(d == 0),
                        stop=(d == DP_TILES - 1),
                        perf_mode=_DR,
                    )
                    if d == DP_TILES - 1:
                        mm.then_inc(mm_sem, 1)
            # pass 2: groups 8-15 reuse banks 0-7 once the epilogue has
            # drained the pass-1 group from that bank (P10: concurrent
            # PE-write + engine-read of one PSUM bank is fatal, so this wait
            # is load-bearing, not just WAR ordering).  Even banks are freed
            # by the DVE, odd banks by the Act engine.
            for g in range(GP1, G):
                m, h = _g_mh(g)
                if g % 2 == 0:
                    tensor.wait_ge(dve_e, (g - 8) // 2 + 1)
                else:
                    tensor.wait_ge(ob_sem, (g - 8) // 2 + 1)
                for d in range(DP_TILES):
                    mm = nc.tensor.matmul(
                        ps[g % 8][:],
                        ct_sb[d][:, :, m * P : (m + 1) * P],
                        xt_sb[d][:, :, h * NF : (h + 1) * NF],
                        start=(d == 0),
                        stop=(d == DP_TILES - 1),
                        perf_mode=_DR,
                    )
                mm.then_inc(mm_sem, 1)

        @block.vector
        def _(vector):
            vector.wait_ge(const_sem, 16)  # ncsq present
            vector.wait_ge(bc_sem, 1)      # nxsq broadcast done
            n_tt = 0

            def tt_finish(g, lo, hi):
                # odd-group step 2: ot = tmp + nxsq.  All-fp16 SBUF operands
                # -> DVE 2x perf mode (~327ns vs 658ns for the PSUM STT).
                nonlocal n_tt
                buf = n_tt % N_TMP
                vector.wait_ge(ob_sem, n_tt + 1)
                _, h = _g_mh(g)
                nc.vector.tensor_add(
                    ot_sb[:, g * NF + lo : g * NF + hi],
                    tmp_sb[:, buf, 0 : hi - lo],
                    nxsq_sb[:, h * NF + lo : h * NF + hi],
                ).then_inc(od_sem, 1)
                n_tt += 1

            for g in range(0, G - 1):
                if g % 2 == 0:
                    m, h = _g_mh(g)
                    vector.wait_ge(mm_sem, g + 1)
                    nc.vector.scalar_tensor_tensor(
                        ot_sb[:, g * NF : (g + 1) * NF],
                        ps[g % 8][:],
                        ncsq_sb[:, m : m + 1],
                        nxsq_sb[:, h * NF : (h + 1) * NF],
                        op0=mybir.AluOpType.add,
                        op1=mybir.AluOpType.add,
                    ).then_inc(dve_e, 1)
                else:
                    tt_finish(g, 0, NF)
            tt_finish(G - 1, 0, HNF)
            tt_finish(G - 1, HNF, NF)

    nc.compile()
    return nc


def _get_nc():
    if not hasattr(_cache, "nc"):
        _cache.nc = _build_nc()
    return _cache.nc


def _to_dr_layout(a_t):
    """[D, F] -> [DP_TILES, P, 2, F]: d = dp*256 + i*128 + p."""
    F = a_t.shape[1]
    return np.ascontiguousarray(
        a_t.reshape(DP_TILES, 2, P, F).transpose(0, 2, 1, 3)
    )


def kernel(inputs, centers, _trace=False):
    inputs = np.asarray(inputs, dtype=np.float32)
    centers = np.asarray(centers, dtype=np.float32)

    csq = np.sum(centers.astype(np.float64) ** 2, axis=1)
    xsq = np.sum(inputs.astype(np.float64) ** 2, axis=1)

    ct8 = _to_dr_layout(centers.T.astype(_NP_DT))
    xt8_full = (2.0 * inputs).T.astype(_NP_DT)  # [D, N]
    ncsq = np.ascontiguousarray((-csq).reshape(M_TILES, P).T.astype(np.float32))

    in_maps = []
    for i in range(N_CORES):
        sl = slice(i * NSH, (i + 1) * NSH)
        in_maps.append(
            {
                "ct": ct8,
                "xt": _to_dr_layout(xt8_full[:, sl]),
                "ncsq": ncsq,
                "nxrow": np.ascontiguousarray(
                    (-xsq[sl]).reshape(1, NSH).astype(np.float16)
                ),
            }
        )

    nc = _get_nc()
    try:
        res = run_bass_kernel_spmd(
            nc, in_maps, core_ids=list(range(N_CORES)), trace=_trace
        )
    except ModuleNotFoundError:
        # NTFF trace glue is absent in some images; rerun without tracing
        res = run_bass_kernel_spmd(
            nc, in_maps, core_ids=list(range(N_CORES)), trace=False
        )
    if _trace:
        kernel.last_results = res
    return np.concatenate(
        [np.asarray(r["out"]).astype(np.float32) for r in res.results], axis=1
    )


# revision 9
# speedup vs baseline: 1.0444x; 1.0444x over previous
"""Trainium2 Bass kernel for nn_CentersDistance (retrieval_knn).

logits[k, n] = -||centers[k] - inputs[n]||^2
             = 2*(centers @ inputs.T)[k, n] - ||centers[k]||^2 - ||inputs[n]||^2

Strategy (8 NeuronCores, data-parallel over the N=8192 inputs):
  * host: transpose both operands so the contraction dim D lands on the SBUF
    partition axis, fold the factor 2 into the inputs, quantize both to
    fp8e4m3 (TRN float8e4), and precompute the norm terms exactly in float64.
  * device (per core): a 1024x1024x1024 matmul in fp8 with DoubleRow perf
    mode: each InstMatmult consumes TWO 128-deep contraction tiles laid out
    as [128, 2, free] (2 rows/cycle on the PE = 157 TF/s, 2x the bf16 rate),
    so the whole GEMM is 64 matmul instructions instead of 128.  PSUM
    accumulation stays fp32.
  * the PSUM->SBUF epilogue (add -||c||^2 per-partition and -||x||^2
    broadcast row, emit bf16) is the serial tail bottleneck (~740ns per
    [128, 512] group on the DVE), so it is split across THREE engines:
      - even groups: DVE scalar_tensor_tensor (one op per group);
      - odd groups: Activation engine adds the per-partition -||c||^2 via
        an Identity-activation bias read straight from PSUM (this also
        frees the PSUM bank for pass 2), then GpSimd adds the -||x||^2 row
        (tensor_add, SBUF-only - GpSimd cannot touch PSUM).
  * -||x||^2 is shipped as a single [1, 1024] fp32 row (4 KB) and broadcast
    to all 128 partitions on-chip by GpSimd (partition_broadcast), replacing
    the baseline's 512 KB host-broadcast load.
  * raw Block/semaphore implementation (not Tile), same skeleton as the
    bf16 baseline: two HW-DGE queues (Sync: xt, Scalar: ct) with one
    semaphore per d-pair; PE warmup matmuls bridge the NRT preamble until
    the first tile pair lands (~10.5us) and keep the HAM clock ramp alive
    (an idle PE resets it - observed: a 4us tile stall restarted the ramp
    and cost 13 matmuls at 1.2GHz); pass 1 (m-tiles 0-3) runs d outermost
    to pace with the streaming loads across 8 PSUM banks; pass 2 (m-tiles
    4-7) runs d innermost so each output group retires early and its
    epilogue + store overlap the remaining matmuls.
  * stores pair adjacent groups (same m-tile -> contiguous in out) into
    single [128, 1024] bf16 DMAs (2 KB/partition lines) alternating between
    the two queues; the last group is split in half across both queues to
    shorten the tail.  Output is bf16 (halves store traffic vs fp32; host
    converts back).

Accuracy: the exact f64 norm terms dominate the logits; fp8 cross term +
bf16 store measured absmax/scale 7.3e-3 vs the 2e-2 gate (bf16 baseline:
3.3e-4 at 45us, kept in kernel_bf16_baseline.py as fallback).
"""

import threading
from contextlib import ExitStack

import numpy as np
import ml_dtypes

import concourse.mybir as mybir
from concourse import bacc
from concourse.bass_utils import run_bass_kernel_spmd

N_CORES = 8
N, K, D = 8192, 1024, 1024
NSH = N // N_CORES  # per-core slab of inputs
P = 128             # SBUF partitions
NF = 512            # matmul moving free dim (one fp32 PSUM bank)

DP_TILES = D // (2 * P)  # 4 double-row contraction tiles (256 deep each)
M_TILES = K // P         # 8 center tiles
H_TILES = NSH // NF      # 2 moving-dim tiles

G = M_TILES * H_TILES  # 16 output groups of [128, 512]
GP1 = 8                # groups 0-7 -> pass 1 (m-tiles 0-3), banks 0-7
N_WU = 10              # PE warm-up matmuls (bridge preamble-end -> first tile)
N_TMP = 4              # fp32 staging buffers for the Act->GpSimd odd path

_DT = mybir.dt.float8e4
_NP_DT = ml_dtypes.float8_e4m3
_OUT_DT = mybir.dt.float16
_DR = mybir.MatmulPerfMode.DoubleRow

_cache = threading.local()


def _g_mh(g):
    return g // H_TILES, g % H_TILES


def _build_nc():
    nc = bacc.Bacc(
        "TRN2", target_bir_lowering=False, debug=False, num_devices=N_CORES
    )
    ct = nc.dram_tensor("ct", [DP_TILES, P, 2, K], _DT, kind="ExternalInput").ap()
    xt = nc.dram_tensor("xt", [DP_TILES, P, 2, NSH], _DT, kind="ExternalInput").ap()
    ncsq = nc.dram_tensor(
        "ncsq", [P, M_TILES], mybir.dt.float32, kind="ExternalInput"
    ).ap()
    nxrow = nc.dram_tensor(
        "nxrow", [1, NSH], mybir.dt.float16, kind="ExternalInput"
    ).ap()
    out = nc.dram_tensor("out", [K, NSH], _OUT_DT, kind="ExternalOutput").ap()

    out_r = out.rearrange("(m p) n -> m p n", p=P)

    HNF = NF // 2
    ODD = list(range(1, G - 1, 2))  # odd groups 1..13 via Act+GpSimd
    # group 15 is drained in two halves, also via Act+GpSimd (tail latency)

    with (
        nc.sbuf_tensor("wu_sb", [P, 2, NF], _DT) as wu_sb,
        nc.sbuf_tensor("ncsq_sb", [P, M_TILES], mybir.dt.float32) as ncsq_sb,
        nc.sbuf_tensor("nxrow_sb", [1, NSH], mybir.dt.float16) as nxrow_sb,
        nc.sbuf_tensor("nxsq_sb", [P, NSH], mybir.dt.float16) as nxsq_sb,
        nc.sbuf_tensor("tmp_sb", [P, N_TMP, NF], mybir.dt.float16) as tmp_sb,
        nc.sbuf_tensor("ot_sb", [P, G * NF], _OUT_DT) as ot_sb,
        ExitStack() as stack,
        nc.semaphore("row_sem") as row_sem,
        nc.semaphore("const_sem") as const_sem,
        nc.semaphore("bc_sem") as bc_sem,
        nc.semaphore("mm_sem") as mm_sem,
        nc.semaphore("dve_e") as dve_e,   # even group drained+done (DVE)
        nc.semaphore("ob_sem") as ob_sem, # odd group PSUM drained (Act)
        nc.semaphore("od_sem") as od_sem, # odd group data ready (GpSimd)
        nc.semaphore("dma_out") as dma_out,
        nc.Block() as block,
    ):
        d_sems = [
            stack.enter_context(nc.semaphore(f"d_sem{i}")) for i in range(DP_TILES)
        ]
        ct_sb = [
            stack.enter_context(nc.sbuf_tensor(f"ct_sb{d}", [P, 2, K], _DT))
            for d in range(DP_TILES)
        ]
        xt_sb = [
            stack.enter_context(nc.sbuf_tensor(f"xt_sb{d}", [P, 2, NSH], _DT))
            for d in range(DP_TILES)
        ]
        ps = [
            stack.enter_context(nc.psum_tensor(f"ps{b}", [P, NF], mybir.dt.float32))
            for b in range(8)
        ]

        # store schedule: pair adjacent groups (same m-tile -> contiguous in
        # out) into one [128, 1024] bf16 DMA with 2KB/partition lines.
        # Sync queue: pairs 0,2,4,6 + group 14 + first half of group 15.
        # Scalar queue: pairs 1,3,5 + second half of group 15.
        def pair_store(eng, gp):
            g0 = 2 * gp
            m, _ = _g_mh(g0)
            eng.wait_ge(dve_e, gp + 1)
            eng.wait_ge(od_sem, gp + 1)
            eng.dma_start(
                out_r[m][:],
                ot_sb[:, g0 * NF : (g0 + 2) * NF],
            ).then_inc(dma_out, 16)

        N_STORES = 7 + 1 + 2  # 7 pairs + group 14 + two halves of group 15

        @block.sync
        def _(sync):
            # consts first: 6KB, and the GpSimd broadcast wants nxrow early
            sync.dma_start(nxrow_sb[:], nxrow).then_inc(row_sem, 16)
            sync.dma_start(ncsq_sb[:], ncsq).then_inc(const_sem, 16)
            # xt tiles 0-2 on the Sync HW-DGE queue; ct 0-2 in parallel on
            # the Scalar queue; the dp3 pair rides GpSimd's software-DGE
            # ring as a third HBM stream (loads are the critical resource:
            # the pass-1 stop round cannot run until the last tile lands)
            for d in (0, 2, 3):
                sync.dma_start(xt_sb[d][:], xt[d]).then_inc(d_sems[d], 16)
            for gp in (0, 2, 4, 6):
                pair_store(sync, gp)
            # group 14 whole
            m, h = _g_mh(G - 2)
            sync.wait_ge(dve_e, 8)
            sync.dma_start(
                out_r[m][:, h * NF : (h + 1) * NF],
                ot_sb[:, (G - 2) * NF : (G - 1) * NF],
            ).then_inc(dma_out, 16)
            # first half of group 15
            m, h = _g_mh(G - 1)
            sync.wait_ge(od_sem, 8)
            sync.dma_start(
                out_r[m][:, h * NF : h * NF + HNF],
                ot_sb[:, (G - 1) * NF : (G - 1) * NF + HNF],
            ).then_inc(dma_out, 16)
            sync.wait_ge(dma_out, N_STORES * 16)

        @block.scalar
        def _(scalar):
            for d in (0, 2, 3):
                scalar.dma_start(ct_sb[d][:], ct[d]).then_inc(d_sems[d], 16)
            # odd-group epilogue step 1 on the Activation engine:
            # tmp = Identity(ps + ncsq_bias).  Reads PSUM (which GpSimd
            # cannot), so this is also what frees the bank for pass 2.
            scalar.wait_ge(const_sem, 16)  # ncsq present
            n_act = 0

            def act_drain(g, lo, hi):
                nonlocal n_act
                buf = n_act % N_TMP
                if n_act >= N_TMP:
                    # buffer reuse: GpSimd must have consumed its previous
                    # occupant
                    scalar.wait_ge(od_sem, n_act - N_TMP + 1)
                m, _ = _g_mh(g)
                nc.scalar.add(
                    tmp_sb[:, buf, 0 : hi - lo],
                    ps[g % 8][:, lo:hi],
                    ncsq_sb[:, m : m + 1],
                ).then_inc(ob_sem, 1)
                n_act += 1

            # interleave the Scalar-queue stores so they dispatch as soon
            # as their groups are ready without stalling later act ops
            for j, g in enumerate(ODD):
                scalar.wait_ge(mm_sem, g + 1)
                act_drain(g, 0, NF)
                if g == 5:
                    pair_store(scalar, 1)   # groups (2,3)
                elif g == 9:
                    pair_store(scalar, 3)   # groups (6,7)
                elif g == 13:
                    pair_store(scalar, 5)   # groups (10,11)
            # group 15 in two halves
            scalar.wait_ge(mm_sem, G)
            act_drain(G - 1, 0, HNF)
            act_drain(G - 1, HNF, NF)
            # second half of group 15
            m, h = _g_mh(G - 1)
            scalar.wait_ge(od_sem, 9)
            scalar.dma_start(
                out_r[m][:, h * NF + HNF : (h + 1) * NF],
                ot_sb[:, (G - 1) * NF + HNF : G * NF],
            ).then_inc(dma_out, 16)

        @block.gpsimd
        def _(gpsimd):
            # dp1 rides the software-DGE ring: it spins up ~2us after the HW
            # queues and lands its pair second, right when pass 1 wants it
            gpsimd.dma_start(xt_sb[1][:], xt[1]).then_inc(d_sems[1], 16)
            gpsimd.dma_start(ct_sb[1][:], ct[1]).then_inc(d_sems[1], 16)
            gpsimd.wait_ge(row_sem, 16)
            nc.gpsimd.partition_broadcast(nxsq_sb[:], nxrow_sb[:]).then_inc(
                bc_sem, 1
            )

        @block.tensor
        def _(tensor):
            # warm-up: keep the PE busy (and the HAM clock ramp alive) from
            # preamble-end until the first ct/xt tile pair lands.  wu_sb is
            # deliberately uninitialized - the products are never read.
            # Bank 7 is rewritten with start=True by group 7's first matmul
            # ~8 matmuls later.
            for _ in range(N_WU):
                nc.tensor.matmul(
                    ps[GP1 - 1][:],
                    wu_sb[:, :, 0:P],
                    wu_sb[:, :, :],
                    start=True,
                    stop=True,
                    perf_mode=_DR,
                )
            # pass 1: groups 0-7 accumulate in banks 0-7, d outermost so
            # matmuls pace with the streaming loads
            for d in range(DP_TILES):
                tensor.wait_ge(d_sems[d], 32)
                for g in range(GP1):
                    m, h = _g_mh(g)
                    mm = nc.tensor.matmul(
                        ps[g][:],
                        ct_sb[d][:, :, m * P : (m + 1) * P],
                        xt_sb[d][:, :, h * NF : (h + 1) * NF],
                        start=(d == 0),
                        stop=(d == DP_TILES - 1),
                        perf_mode=_DR,
                    )
                    if d == DP_TILES - 1:
                        mm.then_inc(mm_sem, 1)
            # pass 2: groups 8-15 reuse banks 0-7 once the epilogue has
            # drained the pass-1 group from that bank (P10: concurrent
            # PE-write + engine-read of one PSUM bank is fatal, so this wait
            # is load-bearing, not just WAR ordering).  Even banks are freed
            # by the DVE, odd banks by the Act engine.
            for g in range(GP1, G):
                m, h = _g_mh(g)
                if g % 2 == 0:
                    tensor.wait_ge(dve_e, (g - 8) // 2 + 1)
                else:
                    tensor.wait_ge(ob_sem, (g - 8) // 2 + 1)
                for d in range(DP_TILES):
                    mm = nc.tensor.matmul(
                        ps[g % 8][:],
                        ct_sb[d][:, :, m * P : (m + 1) * P],
                        xt_sb[d][:, :, h * NF : (h + 1) * NF],
                        start=(d == 0),
                        stop=(d == DP_TILES - 1),
                        perf_mode=_DR,
                    )
                mm.then_inc(mm_sem, 1)

        @block.vector
        def _(vector):
            vector.wait_ge(const_sem, 16)  # ncsq present
            vector.wait_ge(bc_sem, 1)      # nxsq broadcast done
            n_tt = 0

            def tt_finish(g, lo, hi):
                # odd-group step 2: ot = tmp + nxsq.  All-fp16 SBUF operands
                # -> DVE 2x perf mode (~327ns vs 658ns for the PSUM STT).
                nonlocal n_tt
                buf = n_tt % N_TMP
                vector.wait_ge(ob_sem, n_tt + 1)
                _, h = _g_mh(g)
                nc.vector.tensor_add(
                    ot_sb[:, g * NF + lo : g * NF + hi],
                    tmp_sb[:, buf, 0 : hi - lo],
                    nxsq_sb[:, h * NF + lo : h * NF + hi],
                ).then_inc(od_sem, 1)
                n_tt += 1

            for g in range(0, G - 1):
                if g % 2 == 0:
                    m, h = _g_mh(g)
                    vector.wait_ge(mm_sem, g + 1)
                    nc.vector.scalar_tensor_tensor(
                        ot_sb[:, g * NF : (g + 1) * NF],
                        ps[g % 8][:],
                        ncsq_sb[:, m : m + 1],
                        nxsq_sb[:, h * NF : (h + 1) * NF],
                        op0=mybir.AluOpType.add,
                        op1=mybir.AluOpType.add,
                    ).then_inc(dve_e, 1)
                else:
                    tt_finish(g, 0, NF)
            tt_finish(G - 1, 0, HNF)
            tt_finish(G - 1, HNF, NF)

    nc.compile()
    return nc


def _get_nc():
    if not hasattr(_cache, "nc"):
        _cache.nc = _build_nc()
    return _cache.nc


def _to_dr_layout(a_t):
    """[D, F] -> [DP_TILES, P, 2, F]: d = dp*256 + i*128 + p."""
    F = a_t.shape[1]
    return np.ascontiguousarray(
        a_t.reshape(DP_TILES, 2, P, F).transpose(0, 2, 1, 3)
    )


def kernel(inputs, centers, _trace=False):
    inputs = np.asarray(inputs, dtype=np.float32)
    centers = np.asarray(centers, dtype=np.float32)

    csq = np.sum(centers.astype(np.float64) ** 2, axis=1)
    xsq = np.sum(inputs.astype(np.float64) ** 2, axis=1)

    ct8 = _to_dr_layout(centers.T.astype(_NP_DT))
    xt8_full = (2.0 * inputs).T.astype(_NP_DT)  # [D, N]
    ncsq = np.ascontiguousarray((-csq).reshape(M_TILES, P).T.astype(np.float32))

    in_maps = []
    for i in range(N_CORES):
        sl = slice(i * NSH, (i + 1) * NSH)
        in_maps.append(
            {
                "ct": ct8,
                "xt": _to_dr_layout(xt8_full[:, sl]),
                "ncsq": ncsq,
                "nxrow": np.ascontiguousarray(
                    (-xsq[sl]).reshape(1, NSH).astype(np.float16)
                ),
            }
        )

    nc = _get_nc()
    try:
        res = run_bass_kernel_spmd(
            nc, in_maps, core_ids=list(range(N_CORES)), trace=_trace
        )
    except ModuleNotFoundError:
        # NTFF trace glue is absent in some images; rerun without tracing
        res = run_bass_kernel_spmd(
            nc, in_maps, core_ids=list(range(N_CORES)), trace=False
        )
    if _trace:
        kernel.last_results = res
    return np.concatenate(
        [np.asarray(r["out"]).astype(np.float32) for r in res.results], axis=1
    )
